# revision 12
# baseline (speedup 1.0000x reference)
"""Trainium2 Bass kernel for a 16-head attention block (B=2, S=2048, D=1024).

The reference discards its softmax, so attention reduces to
(Q K^T / sqrt(dk)) V = Q (K^T V) / sqrt(dk): per head only a 64x64 Gram
matrix G_h = K_h^T V_h is needed, never the SxS score matrix.

Sharding (tensor parallel over heads, data parallel over batch): each of the
8 cores owns one batch and 4 of the 16 heads — the matching 256-column slice
of w_q/w_k/w_v and 256-row slice of w_o — over the full 2048-token sequence.
Every core is fully independent (no device collective); each returns its
w_o partial product and the host sums the four head-group partials per batch
(+ b_o) while gathering, which is the unshard step for TP sharding.

Schedule (v2): Q projection runs FIRST (it is weight-stationary and its
consumers sit at the very end of the kernel), K second, V last with the Gram
accumulation interleaved into V's evictions, so the output stage launches as
soon as the last xv chunk lands instead of waiting for the last xq chunk.
No warmup matmuls: the first real d-outer chain sweep is dense enough to
warm the HAM clock gate while the input DMA stream paces the start.
"""

import sys

sys.path.insert(0, "/opt/trn_rl_repo")

import numpy as np
import ml_dtypes

import concourse.bacc as bacc
import concourse.tile as tile
import concourse.mybir as mybir
from concourse import bass_utils

B, S, D, H, DK = 2, 2048, 1024, 16, 64
NCORES = 8
HG = H // (NCORES // B)   # 4 heads per core
FH = HG * DK              # 256 head-features per core
NT = S // 128             # 16 sequence tiles
ND = D // 128             # 8 input-feature chunks
NPAIR = FH // 128         # 2 head pairs (2 heads = 128 features)
NSC = S // 512            # 4 sequence chunks of 512

DT = mybir.dt.bfloat16
NP_DT = ml_dtypes.bfloat16
F32 = mybir.dt.float32

_cache = {}


def _build():
    nc = bacc.Bacc("TRN2", target_bir_lowering=False, debug=False,
                   num_devices=NCORES)

    xqT = nc.dram_tensor("xqT", [D, S], DT, kind="ExternalInput")
    xkT = nc.dram_tensor("xkT", [D, S], DT, kind="ExternalInput")
    xvT = nc.dram_tensor("xvT", [D, S], DT, kind="ExternalInput")
    wqT = nc.dram_tensor("wqT", [D, FH], DT, kind="ExternalInput")
    wkT = nc.dram_tensor("wkT", [D, FH], DT, kind="ExternalInput")
    wvT = nc.dram_tensor("wvT", [D, FH], DT, kind="ExternalInput")
    woT = nc.dram_tensor("woT", [FH, D], DT, kind="ExternalInput")
    bk_rep = nc.dram_tensor("bk_rep", [128, FH], DT, kind="ExternalInput")
    bv_rep = nc.dram_tensor("bv_rep", [128, FH], DT, kind="ExternalInput")
    bqT = nc.dram_tensor("bqT", [128, NPAIR], F32, kind="ExternalInput")
    out_h = nc.dram_tensor("out", [S, D], DT, kind="ExternalOutput")

    add = mybir.AluOpType.add

    with tile.TileContext(nc) as tc:
        with (
            tc.tile_pool(name="sb", bufs=1) as sb,
            tc.tile_pool(name="ps", bufs=8, space="PSUM") as ps,
        ):
            # --- SBUF allocations
            xq_sb = sb.tile([128, ND * S], DT, name="xq_sb", tag="xq_sb")
            xk_sb = sb.tile([128, ND * S], DT, name="xk_sb", tag="xk_sb")
            xv_sb = sb.tile([128, ND * S], DT, name="xv_sb", tag="xv_sb")
            wq_sb = sb.tile([128, ND * FH], DT, name="wq_sb", tag="wq_sb")
            wk_sb = sb.tile([128, ND * FH], DT, name="wk_sb", tag="wk_sb")
            wv_sb = sb.tile([128, ND * FH], DT, name="wv_sb", tag="wv_sb")
            wo_sb = sb.tile([128, NPAIR * D], DT, name="wo_sb", tag="wo_sb")
            bk_sb = sb.tile([128, FH], DT, name="bk_sb", tag="bk_sb")
            bv_sb = sb.tile([128, FH], DT, name="bv_sb", tag="bv_sb")
            bq_sb = sb.tile([128, NPAIR], F32, name="bq_sb", tag="bq_sb")
            K_sb = sb.tile([128, NT * FH], DT, name="K_sb", tag="K_sb")
            V_sb = sb.tile([128, NT * FH], DT, name="V_sb", tag="V_sb")
            QT_sb = sb.tile([128, NPAIR * S], DT, name="QT_sb", tag="QT_sb")
            G_sb = sb.tile([128, NPAIR * 128], F32, name="G_sb", tag="G_sb")
            Gbd = sb.tile([128, NPAIR * 128], DT, name="Gbd", tag="Gbd")
            wGO_sb = sb.tile([128, NPAIR * D], DT, name="wGO_sb",
                             tag="wGO_sb")

            # --- input DMAs, in consumption order, all on the sync ring.
            # bq first (needed by the early Q evictions); bk/bv/wo slot in
            # after the xk stream, ahead of their consumers.
            nc.sync.dma_start(out=bq_sb[:], in_=bqT[:, :])
            for d in range(ND):
                nc.sync.dma_start(out=wq_sb[:, d * FH:(d + 1) * FH],
                                  in_=wqT[d * 128:(d + 1) * 128, :])
                nc.sync.dma_start(out=xq_sb[:, d * S:(d + 1) * S],
                                  in_=xqT[d * 128:(d + 1) * 128, :])
            for d in range(ND):
                nc.sync.dma_start(out=wk_sb[:, d * FH:(d + 1) * FH],
                                  in_=wkT[d * 128:(d + 1) * 128, :])
                nc.sync.dma_start(out=xk_sb[:, d * S:(d + 1) * S],
                                  in_=xkT[d * 128:(d + 1) * 128, :])
            nc.sync.dma_start(out=bk_sb[:], in_=bk_rep[:, :])
            nc.sync.dma_start(out=bv_sb[:], in_=bv_rep[:, :])
            for a in range(NPAIR):
                nc.sync.dma_start(out=wo_sb[:, a * D:(a + 1) * D],
                                  in_=woT[a * 128:(a + 1) * 128, :])
            for d in range(ND):
                nc.sync.dma_start(out=wv_sb[:, d * FH:(d + 1) * FH],
                                  in_=wvT[d * 128:(d + 1) * 128, :])
                nc.sync.dma_start(out=xv_sb[:, d * S:(d + 1) * S],
                                  in_=xvT[d * 128:(d + 1) * 128, :])

            nc.vector.memset(G_sb[:], 0.0)

            # --- Q projection, all-d-outer: 8 chains (2 head-pairs x 4
            # sequence chunks), one full PSUM bank each.  Runs cold at the
            # very start (no warmup): the first d-step's ~3.4us of cold
            # matmuls is exactly the HAM warm window, and the xq DMA stream
            # paces the loop anyway.
            qchains = [ps.tile([128, 512], F32, name=f"pq{qb}{sc}",
                               tag="proj")
                       for qb in range(NPAIR) for sc in range(NSC)]
            for d in range(ND):
                for qb in range(NPAIR):
                    for sc in range(NSC):
                        nc.tensor.matmul(
                            qchains[qb * NSC + sc][:],
                            wq_sb[:, d * FH + qb * 128:d * FH + qb * 128 + 128],
                            xq_sb[:, d * S + sc * 512:d * S + sc * 512 + 512],
                            start=(d == 0), stop=(d == ND - 1))

            def q_evict(qb, sc):
                p = qchains[qb * NSC + sc]
                dst = QT_sb[:, qb * S + sc * 512:qb * S + sc * 512 + 512]
                if (qb * NSC + sc) % 2 == 0:
                    nc.vector.tensor_scalar(
                        out=dst, in0=p[:], scalar1=bq_sb[:, qb:qb + 1],
                        scalar2=None, op0=add)
                else:
                    nc.scalar.activation(
                        dst, p[:], mybir.ActivationFunctionType.Identity,
                        bias=bq_sb[:, qb:qb + 1])

            # evict in (sc, qb) order so the first PSUM banks free up for
            # the K chains as early as possible
            for sc in range(NSC):
                for qb in range(NPAIR):
                    q_evict(qb, sc)

            # --- K projection, all-16-tile d-outer: two [128,256] chains
            # packed per [128,512] PSUM bank so the whole sweep fits in 8
            # banks and every d-step is 16 back-to-back matmuls (dense
            # enough to outrun the xk DMA stream).
            kslots = [ps.tile([128, 512], F32, name=f"pk{j}", tag="proj")
                      for j in range(8)]
            # two 256-wide chains share each 2KB PSUM bank; a bank is ONE
            # accumulation group (start=True zeroes the whole 2KB zero
            # region), so only the bank's first MM starts and only its last
            # MM stops.
            for d in range(ND):
                for t in range(NT):
                    half = (t % 2) * 256
                    nc.tensor.matmul(
                        kslots[t // 2][:, half:half + 256],
                        xk_sb[:, d * S + t * 128:d * S + (t + 1) * 128],
                        wk_sb[:, d * FH:(d + 1) * FH],
                        start=(d == 0 and t % 2 == 0),
                        stop=(d == ND - 1 and t % 2 == 1))

            def kv_evict(slot, half, bias_sb, dst_sb, t, eng):
                dst = dst_sb[:, t * FH:(t + 1) * FH]
                if eng == 0:
                    nc.vector.tensor_tensor(
                        out=dst, in0=slot[:, half:half + 256],
                        in1=bias_sb[:], op=add)
                else:
                    # GPSIMD cannot read PSUM: ACT does the PSUM->SBUF cast,
                    # GPSIMD adds the bias in-place in SBUF.
                    nc.scalar.copy(out=dst, in_=slot[:, half:half + 256])
                    nc.gpsimd.tensor_tensor(
                        out=dst, in0=dst, in1=bias_sb[:], op=add)

            for t in range(NT):
                kv_evict(kslots[t // 2], (t % 2) * 256, bk_sb, K_sb, t, t % 2)

            # --- V projection: 14 d-outer chains in 7 banks; the last bank
            # holds the two Gram-accumulation chains (one per head pair).
            # Tiles 14/15 run t-outer after the d-loop.  g_tile(t) follows
            # each V eviction so the Gram matrix closes right behind V.
            pgs = ps.tile([128, 512], F32, name="pgs", tag="proj")

            def g_tile(t):
                # accumulates G' = V^T K into pgs slices; Gbd below keeps
                # only the per-head diagonal blocks (the lhsT for wGO).
                # Both pair-slices share one bank = one accumulation group.
                for pr in range(NPAIR):
                    nc.tensor.matmul(
                        pgs[:, pr * 128:(pr + 1) * 128],
                        V_sb[:, t * FH + pr * 128:t * FH + (pr + 1) * 128],
                        K_sb[:, t * FH + pr * 128:t * FH + (pr + 1) * 128],
                        start=(t == 0 and pr == 0),
                        stop=(t == NT - 1 and pr == NPAIR - 1))

            NVC = 14
            vslots = [ps.tile([128, 512], F32, name=f"pv{j}", tag="proj")
                      for j in range(NVC // 2)]
            for d in range(ND):
                for t in range(NVC):
                    half = (t % 2) * 256
                    nc.tensor.matmul(
                        vslots[t // 2][:, half:half + 256],
                        xv_sb[:, d * S + t * 128:d * S + (t + 1) * 128],
                        wv_sb[:, d * FH:(d + 1) * FH],
                        start=(d == 0 and t % 2 == 0),
                        stop=(d == ND - 1 and t % 2 == 1))
            for t in range(NVC):
                kv_evict(vslots[t // 2], (t % 2) * 256, bv_sb, V_sb, t, t % 2)
                g_tile(t)
            for t in range(NVC, NT):
                p = ps.tile([128, 512], F32, name=f"pvt{t}", tag="proj")
                for d in range(ND):
                    nc.tensor.matmul(
                        p[:, 0:256],
                        xv_sb[:, d * S + t * 128:d * S + (t + 1) * 128],
                        wv_sb[:, d * FH:(d + 1) * FH],
                        start=(d == 0), stop=(d == ND - 1))
                kv_evict(p, 0, bv_sb, V_sb, t, t % 2)
                g_tile(t)

            for pr in range(NPAIR):
                # keep only the per-head diagonal blocks, scaled by 1/sqrt(dk)
                nc.vector.tensor_scalar_mul(
                    G_sb[0:64, pr * 128:pr * 128 + 64],
                    pgs[0:64, pr * 128:pr * 128 + 64], 0.125)
                nc.scalar.activation(
                    G_sb[64:128, pr * 128 + 64:(pr + 1) * 128],
                    pgs[64:128, pr * 128 + 64:(pr + 1) * 128],
                    mybir.ActivationFunctionType.Identity, scale=0.125)
            nc.vector.tensor_copy(out=Gbd[:, 0:128], in_=G_sb[:, 0:128])
            nc.scalar.copy(out=Gbd[:, 128:256], in_=G_sb[:, 128:256])

            # wGO = Gs @ woT_slice, per head-pair block (block-diagonal
            # Gs means no accumulation across pairs)
            for ib in range(NPAIR):
                for o in range(D // 512):
                    pw = ps.tile([128, 512], F32, name=f"pw{ib}{o}",
                                 tag="proj")
                    nc.tensor.matmul(
                        pw[:], Gbd[:, ib * 128:(ib + 1) * 128],
                        wo_sb[:, ib * D + o * 512:ib * D + o * 512 + 512],
                        start=True, stop=True)
                    dst = wGO_sb[:, ib * D + o * 512:ib * D + o * 512 + 512]
                    if (2 * ib + o) % 2 == 0:
                        nc.vector.tensor_copy(out=dst, in_=pw[:])
                    else:
                        nc.scalar.copy(out=dst, in_=pw[:])

            # --- output stage: out = QT^T @ wGO, streamed per sequence tile
            for t in range(NT):
                ot = sb.tile([128, D], DT, name=f"ot{t}", tag="out_t",
                             bufs=4)
                for o in range(D // 512):
                    po = ps.tile([128, 512], F32, name=f"po{t}{o}",
                                 tag="proj")
                    for a in range(NPAIR):
                        nc.tensor.matmul(
                            po[:],
                            QT_sb[:, a * S + t * 128:a * S + t * 128 + 128],
                            wGO_sb[:, a * D + o * 512:a * D + o * 512 + 512],
                            start=(a == 0), stop=(a == NPAIR - 1))
                    if (2 * t + o) % 2 == 0:
                        nc.vector.tensor_copy(
                            out=ot[:, o * 512:o * 512 + 512], in_=po[:])
                    else:
                        nc.scalar.copy(
                            out=ot[:, o * 512:o * 512 + 512], in_=po[:])
                nc.sync.dma_start(out=out_h[t * 128:(t + 1) * 128, :],
                                  in_=ot[:])

    nc.compile()
    return nc


def _prep_in_maps(q, k, v, w_q, b_q, w_k, b_k, w_v, b_v, w_o, b_o):
    q, k, v = (np.asarray(x, np.float32) for x in (q, k, v))
    wqT = np.ascontiguousarray(np.asarray(w_q, np.float32).T).astype(NP_DT)
    wkT = np.ascontiguousarray(np.asarray(w_k, np.float32).T).astype(NP_DT)
    wvT = np.ascontiguousarray(np.asarray(w_v, np.float32).T).astype(NP_DT)
    woT = np.ascontiguousarray(np.asarray(w_o, np.float32).T).astype(NP_DT)
    b_q32 = np.asarray(b_q, np.float32)
    b_k32 = np.asarray(b_k, np.float32)
    b_v32 = np.asarray(b_v, np.float32)

    xT = {}
    for b in range(B):
        xT[b] = (
            np.ascontiguousarray(q[b].T).astype(NP_DT),
            np.ascontiguousarray(k[b].T).astype(NP_DT),
            np.ascontiguousarray(v[b].T).astype(NP_DT),
        )

    in_maps = []
    for c in range(NCORES):
        b, hg = divmod(c, NCORES // B)
        F = slice(hg * FH, (hg + 1) * FH)
        qT_b, kT_b, vT_b = xT[b]
        in_maps.append({
            "xqT": qT_b, "xkT": kT_b, "xvT": vT_b,
            "wqT": np.ascontiguousarray(wqT[:, F]),
            "wkT": np.ascontiguousarray(wkT[:, F]),
            "wvT": np.ascontiguousarray(wvT[:, F]),
            "woT": np.ascontiguousarray(woT[F, :]),
            "bk_rep": np.ascontiguousarray(
                np.broadcast_to(b_k32[F], (128, FH))).astype(NP_DT),
            "bv_rep": np.ascontiguousarray(
                np.broadcast_to(b_v32[F], (128, FH))).astype(NP_DT),
            "bqT": np.ascontiguousarray(b_q32[F].reshape(NPAIR, 128).T),
        })
    return in_maps


def _run(in_maps, trace=False):
    if "nc" not in _cache:
        _cache["nc"] = _build()
    nc = _cache["nc"]
    last_err = None
    for _attempt in range(3):
        try:
            return bass_utils.run_bass_kernel_spmd(
                nc, in_maps, core_ids=list(range(NCORES)), trace=trace)
        except Exception as e:  # transient NRT failures happen under axon
            last_err = e
    raise last_err


def _assemble(res, b_o):
    ncg = NCORES // B
    out = np.empty((B, S, D), np.float32)
    for b in range(B):
        acc = res.results[b * ncg]["out"].astype(np.float32)
        for hg in range(1, ncg):
            acc += res.results[b * ncg + hg]["out"].astype(np.float32)
        acc += np.asarray(b_o, np.float32)[None, :]
        out[b] = acc
    return out


def kernel(q, k, v, w_q, b_q, w_k, b_k, w_v, b_v, w_o, b_o):
    in_maps = _prep_in_maps(q, k, v, w_q, b_q, w_k, b_k, w_v, b_v, w_o, b_o)
    res = _run(in_maps, trace=False)
    return _assemble(res, b_o)


def kernel_traced(q, k, v, w_q, b_q, w_k, b_k, w_v, b_v, w_o, b_o):
    """Same as kernel() but profiles on hardware; returns (out, exec_ns, res)."""
    in_maps = _prep_in_maps(q, k, v, w_q, b_q, w_k, b_k, w_v, b_v, w_o, b_o)
    res = _run(in_maps, trace=True)
    return _assemble(res, b_o), res.exec_time_ns, res


# revision 15
# speedup vs baseline: 1.0059x; 1.0059x over previous
"""Trainium2 Bass kernel for a 16-head attention block (B=2, S=2048, D=1024).

The reference discards its softmax, so attention reduces to
(Q K^T / sqrt(dk)) V = Q (K^T V) / sqrt(dk): per head only a 64x64 Gram
matrix G_h = K_h^T V_h is needed, never the SxS score matrix.

Sharding (tensor parallel over heads, data parallel over batch): each of the
8 cores owns one batch and 4 of the 16 heads — the matching 256-column slice
of w_q/w_k/w_v and 256-row slice of w_o — over the full 2048-token sequence.
Every core is fully independent (no device collective); each returns its
w_o partial product and the host sums the four head-group partials per batch
(+ b_o) while gathering, which is the unshard step for TP sharding.

Schedule (v3): Q projection first (weight-stationary; its consumers sit at
the very end), K second, V last with the Gram accumulation interleaved into
V's evictions, so the output stage launches as soon as the last xv chunk
lands.  K/V biases are folded into each PSUM chain as a rank-1 ones-matmul
(bias row broadcast over the 128 sequence rows), so evictions are plain
dual-engine copies.  All weight tensors arrive as single packed DMAs.
"""

import sys

sys.path.insert(0, "/opt/trn_rl_repo")

import numpy as np
import ml_dtypes

import concourse.bacc as bacc
import concourse.tile as tile
import concourse.mybir as mybir
from concourse import bass_utils

B, S, D, H, DK = 2, 2048, 1024, 16, 64
NCORES = 8
HG = H // (NCORES // B)   # 4 heads per core
FH = HG * DK              # 256 head-features per core
NT = S // 128             # 16 sequence tiles
ND = D // 128             # 8 input-feature chunks
NPAIR = FH // 128         # 2 head pairs (2 heads = 128 features)
NSC = S // 512            # 4 sequence chunks of 512

DT = mybir.dt.bfloat16
NP_DT = ml_dtypes.bfloat16
F32 = mybir.dt.float32

_cache = {}


def _build():
    nc = bacc.Bacc("TRN2", target_bir_lowering=False, debug=False,
                   num_devices=NCORES)

    xqT = nc.dram_tensor("xqT", [D, S], DT, kind="ExternalInput")
    xkT = nc.dram_tensor("xkT", [D, S], DT, kind="ExternalInput")
    xvT = nc.dram_tensor("xvT", [D, S], DT, kind="ExternalInput")
    # weights packed host-side as [128, ND*FH]: col d*FH+c <- wT[d*128+p, c]
    wq_pk = nc.dram_tensor("wq_pk", [128, ND * FH], DT, kind="ExternalInput")
    wk_pk = nc.dram_tensor("wk_pk", [128, ND * FH], DT, kind="ExternalInput")
    wv_pk = nc.dram_tensor("wv_pk", [128, ND * FH], DT, kind="ExternalInput")
    wo_pk = nc.dram_tensor("wo_pk", [128, NPAIR * D], DT, kind="ExternalInput")
    bkv_row = nc.dram_tensor("bkv_row", [1, 2 * FH], DT, kind="ExternalInput")
    bqT = nc.dram_tensor("bqT", [128, NPAIR], F32, kind="ExternalInput")
    out_h = nc.dram_tensor("out", [S, D], DT, kind="ExternalOutput")

    add = mybir.AluOpType.add

    with tile.TileContext(nc) as tc:
        with (
            tc.tile_pool(name="sb", bufs=1) as sb,
            tc.tile_pool(name="ps", bufs=8, space="PSUM") as ps,
        ):
            # --- SBUF allocations
            xq_sb = sb.tile([128, ND * S], DT, name="xq_sb", tag="xq_sb")
            xk_sb = sb.tile([128, ND * S], DT, name="xk_sb", tag="xk_sb")
            xv_sb = sb.tile([128, ND * S], DT, name="xv_sb", tag="xv_sb")
            wq_sb = sb.tile([128, ND * FH], DT, name="wq_sb", tag="wq_sb")
            wk_sb = sb.tile([128, ND * FH], DT, name="wk_sb", tag="wk_sb")
            wv_sb = sb.tile([128, ND * FH], DT, name="wv_sb", tag="wv_sb")
            wo_sb = sb.tile([128, NPAIR * D], DT, name="wo_sb", tag="wo_sb")
            bkv_sb = sb.tile([1, 2 * FH], DT, name="bkv_sb", tag="bkv_sb")
            bq_sb = sb.tile([128, NPAIR], F32, name="bq_sb", tag="bq_sb")
            ones_sb = sb.tile([1, 128], DT, name="ones_sb", tag="ones_sb")
            K_sb = sb.tile([128, NT * FH], DT, name="K_sb", tag="K_sb")
            V_sb = sb.tile([128, NT * FH], DT, name="V_sb", tag="V_sb")
            QT_sb = sb.tile([128, NPAIR * S], DT, name="QT_sb", tag="QT_sb")
            Gbd = sb.tile([128, NPAIR * 128], DT, name="Gbd", tag="Gbd")
            wGO_sb = sb.tile([128, NPAIR * D], DT, name="wGO_sb",
                             tag="wGO_sb")

            nc.vector.memset(ones_sb[:], 1.0)
            nc.vector.memset(Gbd[:], 0.0)

            # --- PE warmup on the zeroed Gbd tile (result never read):
            # keeps the HAM activity window busy through the DMA lead-in so
            # the first real matmuls run at 2.4 GHz.
            wp = ps.tile([128, 512], F32, name="wp", tag="proj")
            for i in range(16):
                nc.tensor.matmul(wp[:, 0:256], Gbd[:, 0:128], Gbd[:],
                                 start=(i == 0), stop=(i == 15))

            # --- input DMAs, in consumption order, all on the sync ring
            nc.sync.dma_start(out=wq_sb[:], in_=wq_pk[:, :])
            for d in range(ND):
                nc.sync.dma_start(out=xq_sb[:, d * S:(d + 1) * S],
                                  in_=xqT[d * 128:(d + 1) * 128, :])
                if d == 0:
                    nc.sync.dma_start(out=bq_sb[:], in_=bqT[:, :])
                    nc.sync.dma_start(out=bkv_sb[:], in_=bkv_row[:, :])
            nc.sync.dma_start(out=wk_sb[:], in_=wk_pk[:, :])
            for d in range(ND):
                nc.sync.dma_start(out=xk_sb[:, d * S:(d + 1) * S],
                                  in_=xkT[d * 128:(d + 1) * 128, :])
            nc.sync.dma_start(out=wv_sb[:], in_=wv_pk[:, :])
            nc.sync.dma_start(out=wo_sb[:], in_=wo_pk[:, :])
            for d in range(ND):
                nc.sync.dma_start(out=xv_sb[:, d * S:(d + 1) * S],
                                  in_=xvT[d * 128:(d + 1) * 128, :])

            # --- Q projection, all-d-outer: 8 chains (2 head-pairs x 4
            # sequence chunks), one full PSUM bank each, paced by the xq
            # stream.
            qchains = [ps.tile([128, 512], F32, name=f"pq{qb}{sc}",
                               tag="proj")
                       for qb in range(NPAIR) for sc in range(NSC)]
            for d in range(ND):
                for qb in range(NPAIR):
                    for sc in range(NSC):
                        nc.tensor.matmul(
                            qchains[qb * NSC + sc][:],
                            wq_sb[:, d * FH + qb * 128:d * FH + qb * 128 + 128],
                            xq_sb[:, d * S + sc * 512:d * S + sc * 512 + 512],
                            start=(d == 0), stop=(d == ND - 1))

            def q_evict(qb, sc):
                p = qchains[qb * NSC + sc]
                dst = QT_sb[:, qb * S + sc * 512:qb * S + sc * 512 + 512]
                if (qb * NSC + sc) % 2 == 0:
                    nc.vector.tensor_scalar(
                        out=dst, in0=p[:], scalar1=bq_sb[:, qb:qb + 1],
                        scalar2=None, op0=add)
                else:
                    nc.scalar.activation(
                        dst, p[:], mybir.ActivationFunctionType.Identity,
                        bias=bq_sb[:, qb:qb + 1])

            # evict in (sc, qb) order so the first PSUM banks free up for
            # the K chains as early as possible
            for sc in range(NSC):
                for qb in range(NPAIR):
                    q_evict(qb, sc)

            # --- K projection, all-16-tile d-outer: two 256-wide chains
            # packed per 2KB PSUM bank.  A bank is ONE accumulation group
            # (start=True zeroes the whole zero region): the bank's first MM
            # is the rank-1 bias seed of its even slice, the last is the
            # odd slice's d=7 matmul.
            def proj_sweep(x_sb, w_sb, bias_off, dst_sb, slots, pfx):
                for j, slot in enumerate(slots):
                    for h in range(2):
                        nc.tensor.matmul(
                            slot[:, h * 256:h * 256 + 256],
                            ones_sb[:],
                            bkv_sb[0:1, bias_off:bias_off + 256],
                            start=(h == 0), stop=False)
                for d in range(ND):
                    for t in range(2 * len(slots)):
                        half = (t % 2) * 256
                        nc.tensor.matmul(
                            slots[t // 2][:, half:half + 256],
                            x_sb[:, d * S + t * 128:d * S + (t + 1) * 128],
                            w_sb[:, d * FH:(d + 1) * FH],
                            start=False,
                            stop=(d == ND - 1 and t % 2 == 1))

            def kv_evict(slot, half, dst_sb, t):
                dst = dst_sb[:, t * FH:(t + 1) * FH]
                if t % 2 == 0:
                    nc.vector.tensor_copy(out=dst,
                                          in_=slot[:, half:half + 256])
                else:
                    nc.scalar.copy(out=dst, in_=slot[:, half:half + 256])

            kslots = [ps.tile([128, 512], F32, name=f"pk{j}", tag="proj")
                      for j in range(8)]
            proj_sweep(xk_sb, wk_sb, 0, K_sb, kslots, "pk")
            for t in range(NT):
                kv_evict(kslots[t // 2], (t % 2) * 256, K_sb, t)

            # --- V projection: 14 d-outer chains in 7 banks; the last bank
            # holds the two Gram-accumulation chains (one per head pair).
            # Tiles 14/15 run t-outer after the d-loop.  g_tile(t) follows
            # each V eviction so the Gram matrix closes right behind V.
            pgs = ps.tile([128, 512], F32, name="pgs", tag="proj")

            def g_tile(t):
                # accumulates G' = V^T K into pgs slices; Gbd keeps only the
                # per-head diagonal blocks (the lhsT for wGO).  Both pair
                # slices share one bank = one accumulation group.
                for pr in range(NPAIR):
                    nc.tensor.matmul(
                        pgs[:, pr * 128:(pr + 1) * 128],
                        V_sb[:, t * FH + pr * 128:t * FH + (pr + 1) * 128],
                        K_sb[:, t * FH + pr * 128:t * FH + (pr + 1) * 128],
                        start=(t == 0 and pr == 0),
                        stop=(t == NT - 1 and pr == NPAIR - 1))

            NVC = 14
            vslots = [ps.tile([128, 512], F32, name=f"pv{j}", tag="proj")
                      for j in range(NVC // 2)]
            proj_sweep(xv_sb, wv_sb, FH, V_sb, vslots, "pv")
            for t in range(NVC):
                kv_evict(vslots[t // 2], (t % 2) * 256, V_sb, t)
                g_tile(t)
            for t in range(NVC, NT):
                p = ps.tile([128, 512], F32, name=f"pvt{t}", tag="proj")
                nc.tensor.matmul(p[:, 0:256], ones_sb[:],
                                 bkv_sb[0:1, FH:2 * FH],
                                 start=True, stop=False)
                for d in range(ND):
                    nc.tensor.matmul(
                        p[:, 0:256],
                        xv_sb[:, d * S + t * 128:d * S + (t + 1) * 128],
                        wv_sb[:, d * FH:(d + 1) * FH],
                        start=False, stop=(d == ND - 1))
                kv_evict(p, 0, V_sb, t)
                g_tile(t)

            # Gbd = per-head diagonal blocks of Gs^T, scaled by 1/sqrt(dk),
            # written straight from PSUM (off-diagonal stays memset-zero).
            for pr in range(NPAIR):
                nc.vector.tensor_scalar_mul(
                    Gbd[0:64, pr * 128:pr * 128 + 64],
                    pgs[0:64, pr * 128:pr * 128 + 64], 0.125)
                nc.scalar.activation(
                    Gbd[64:128, pr * 128 + 64:(pr + 1) * 128],
                    pgs[64:128, pr * 128 + 64:(pr + 1) * 128],
                    mybir.ActivationFunctionType.Identity, scale=0.125)

            # wGO = Gs @ woT_slice, per head-pair block (block-diagonal
            # Gs means no accumulation across pairs)
            for ib in range(NPAIR):
                for o in range(D // 512):
                    pw = ps.tile([128, 512], F32, name=f"pw{ib}{o}",
                                 tag="proj")
                    nc.tensor.matmul(
                        pw[:], Gbd[:, ib * 128:(ib + 1) * 128],
                        wo_sb[:, ib * D + o * 512:ib * D + o * 512 + 512],
                        start=True, stop=True)
                    dst = wGO_sb[:, ib * D + o * 512:ib * D + o * 512 + 512]
                    if (2 * ib + o) % 2 == 0:
                        nc.vector.tensor_copy(out=dst, in_=pw[:])
                    else:
                        nc.scalar.copy(out=dst, in_=pw[:])

            # --- output stage: out = QT^T @ wGO, streamed per sequence tile
            for t in range(NT):
                ot = sb.tile([128, D], DT, name=f"ot{t}", tag="out_t",
                             bufs=4)
                for o in range(D // 512):
                    po = ps.tile([128, 512], F32, name=f"po{t}{o}",
                                 tag="proj")
                    for a in range(NPAIR):
                        nc.tensor.matmul(
                            po[:],
                            QT_sb[:, a * S + t * 128:a * S + t * 128 + 128],
                            wGO_sb[:, a * D + o * 512:a * D + o * 512 + 512],
                            start=(a == 0), stop=(a == NPAIR - 1))
                    if (2 * t + o) % 2 == 0:
                        nc.vector.tensor_copy(
                            out=ot[:, o * 512:o * 512 + 512], in_=po[:])
                    else:
                        nc.scalar.copy(
                            out=ot[:, o * 512:o * 512 + 512], in_=po[:])
                    if t == NT - 1:
                        # split the last tile's writeback so the final DMA
                        # is half-sized and starts one eviction earlier
                        nc.sync.dma_start(
                            out=out_h[t * 128:(t + 1) * 128,
                                      o * 512:o * 512 + 512],
                            in_=ot[:, o * 512:o * 512 + 512])
                if t < NT - 1:
                    nc.sync.dma_start(out=out_h[t * 128:(t + 1) * 128, :],
                                      in_=ot[:])

    nc.compile()
    return nc


def _prep_in_maps(q, k, v, w_q, b_q, w_k, b_k, w_v, b_v, w_o, b_o):
    q, k, v = (np.asarray(x, np.float32) for x in (q, k, v))
    wqT = np.ascontiguousarray(np.asarray(w_q, np.float32).T)
    wkT = np.ascontiguousarray(np.asarray(w_k, np.float32).T)
    wvT = np.ascontiguousarray(np.asarray(w_v, np.float32).T)
    woT = np.ascontiguousarray(np.asarray(w_o, np.float32).T)
    b_q32 = np.asarray(b_q, np.float32)
    b_k32 = np.asarray(b_k, np.float32)
    b_v32 = np.asarray(b_v, np.float32)

    def pack(wT_slice, nblk):
        # [nblk*128, W] -> [128, nblk*W]
        nrow, W = wT_slice.shape
        assert nrow == nblk * 128
        return np.ascontiguousarray(
            wT_slice.reshape(nblk, 128, W).transpose(1, 0, 2).reshape(
                128, nblk * W)).astype(NP_DT)

    xT = {}
    for b in range(B):
        xT[b] = (
            np.ascontiguousarray(q[b].T).astype(NP_DT),
            np.ascontiguousarray(k[b].T).astype(NP_DT),
            np.ascontiguousarray(v[b].T).astype(NP_DT),
        )

    in_maps = []
    for c in range(NCORES):
        b, hg = divmod(c, NCORES // B)
        F = slice(hg * FH, (hg + 1) * FH)
        qT_b, kT_b, vT_b = xT[b]
        in_maps.append({
            "xqT": qT_b, "xkT": kT_b, "xvT": vT_b,
            "wq_pk": pack(wqT[:, F], ND),
            "wk_pk": pack(wkT[:, F], ND),
            "wv_pk": pack(wvT[:, F], ND),
            "wo_pk": pack(woT[F, :], NPAIR),
            "bkv_row": np.concatenate(
                [b_k32[F], b_v32[F]]).reshape(1, 2 * FH).astype(NP_DT),
            "bqT": np.ascontiguousarray(b_q32[F].reshape(NPAIR, 128).T),
        })
    return in_maps


def _run(in_maps, trace=False):
    if "nc" not in _cache:
        _cache["nc"] = _build()
    nc = _cache["nc"]
    last_err = None
    for _attempt in range(3):
        try:
            return bass_utils.run_bass_kernel_spmd(
                nc, in_maps, core_ids=list(range(NCORES)), trace=trace)
        except Exception as e:  # transient NRT failures happen under axon
            last_err = e
    raise last_err


def _assemble(res, b_o):
    ncg = NCORES // B
    out = np.empty((B, S, D), np.float32)
    for b in range(B):
        acc = res.results[b * ncg]["out"].astype(np.float32)
        for hg in range(1, ncg):
            acc += res.results[b * ncg + hg]["out"].astype(np.float32)
        acc += np.asarray(b_o, np.float32)[None, :]
        out[b] = acc
    return out


def kernel(q, k, v, w_q, b_q, w_k, b_k, w_v, b_v, w_o, b_o):
    in_maps = _prep_in_maps(q, k, v, w_q, b_q, w_k, b_k, w_v, b_v, w_o, b_o)
    res = _run(in_maps, trace=False)
    return _assemble(res, b_o)


def kernel_traced(q, k, v, w_q, b_q, w_k, b_k, w_v, b_v, w_o, b_o):
    """Same as kernel() but profiles on hardware; returns (out, exec_ns, res)."""
    in_maps = _prep_in_maps(q, k, v, w_q, b_q, w_k, b_k, w_v, b_v, w_o, b_o)
    res = _run(in_maps, trace=True)
    return _assemble(res, b_o), res.exec_time_ns, res
